# revision 1
# baseline (speedup 1.0000x reference)
"""Trainium2 Bass kernel for BilinearDiscriminator.

Computes sigmoid((x*mask_x) @ W.T @ (y*mask_y).T) for x,y [8192,512],
W [512,512] -> out [8192,8192] fp32, SPMD across 8 NeuronCores.

Sharding: 4x2 (n x m) grid. Core c handles n-slab i = c//2 (2048 rows of x)
and m-slab j = c%2 (4096 rows of y). Inputs are passed transposed (d-major)
so both matmuls contract over the SBUF partition dim without on-chip
transposes:
  phase 1: xdT = xT*mxT                      (DVE)
  phase 2: xtT[k,n] = sum_d WT[d,k]*xdT[d,n] (PE, accumulate 4 d-chunks)
  phase 3: ydT = yT*myT                      (DVE)
  phase 4: logits[n,m] = sum_k xtT[k,n]*ydT[k,m] -> sigmoid -> out (PE+ACT)
"""

import os
import sys

sys.path.insert(0, "/opt/trn_rl_repo")

import numpy as np

import concourse.bass as bass
import concourse.mybir as mybir
import concourse.tile as tile
from concourse import bacc
from concourse.bass_utils import run_bass_kernel_spmd

P = 128
N, M, D = 8192, 8192, 512
GRID_N, GRID_M = 4, 2
N_LOC = N // GRID_N  # 2048
M_LOC = M // GRID_M  # 4096
DC = D // P  # 4 chunks of the contraction dims

F32 = mybir.dt.float32
# matmul compute dtype: "float32" (exact, 4 cyc/row), "float32r" (fast fp32,
# 1 cyc/row), "bfloat16" (1 cyc/row, lossy operands)
MM1_DT = os.environ.get("MM1_DT", "float32r")
MM2_DT = os.environ.get("MM2_DT", "float32r")
# output storage dtype: a 2-byte output halves the dominant DMA-write
# traffic (32->16 MiB/core), making the kernel PE-bound instead of
# DMA-bound; the host upcasts to fp32. float16 over bfloat16: sigmoid
# outputs are in [0,1] where f16's range is safe and its 10 mantissa bits
# (vs 8) cut the output-rounding error 4x (rel ~1.4e-4 vs ~5.5e-4).
OUT_DT = os.environ.get("OUT_DT", "float16")
# ablation knobs (experiments only; all default off)
_ABL_NO_OUT = bool(int(os.environ.get("ABL_NO_OUT", "0")))
_ABL_NO_MM2 = bool(int(os.environ.get("ABL_NO_MM2", "0")))
_ABL_NO_IN = bool(int(os.environ.get("ABL_NO_IN", "0")))
_OUTP_BUFS = int(os.environ.get("OUTP_BUFS", "20"))
_STAGE_BUFS = int(os.environ.get("STAGE_BUFS", "4"))
_PSUM2_BUFS = int(os.environ.get("PSUM2_BUFS", "3"))
_MB = int(os.environ.get("MB_SIZE", "1024"))


def _store_dt(mm_dt: str):
    return mybir.dt.bfloat16 if mm_dt == "bfloat16" else F32


def _mm_view(ap: bass.AP, mm_dt: str) -> bass.AP:
    if mm_dt == "float32r":
        return ap.bitcast(mybir.dt.float32r)
    return ap


def _round_out(ap: bass.AP, mm_dt: str) -> bass.AP:
    # fp32r matmul operands must be produced pre-rounded: give the producing
    # instruction an fp32r-typed output view so walrus emits the rounding.
    if mm_dt == "float32r":
        return ap.bitcast(mybir.dt.float32r)
    return ap


def _build(mask_u8: bool = True):
    """Build the SPMD program.

    mask_u8=True: masks are uint8 {0,1}; the dropout scale (product of both
    masks' nonzero values) is applied via the sigmoid's scale input "sc".
    (Bit-packing the masks was tried and is net-negative: the 8-way unpack
    ops' per-instruction overheads delay matmul2's start by more than the
    ~7 us of DMA saved.)
    mask_u8=False: masks are arbitrary fp32, applied directly; sc still
    feeds the sigmoid scale (host passes 1.0).
    """
    # the in-place mask-multiply path assumes fp32-sized operand tiles
    assert MM1_DT != "bfloat16" and MM2_DT != "bfloat16", (
        "bf16 operand storage not supported by the in-place load path"
    )
    nc = bacc.Bacc("TRN2", target_bir_lowering=False, debug=False)

    U8 = mybir.dt.uint8
    MDT = U8 if mask_u8 else F32
    # f32 masks need bigger staging tiles; shrink buffering to fit SBUF
    outp_bufs = _OUTP_BUFS if mask_u8 else min(_OUTP_BUFS, 8)
    stage_bufs = _STAGE_BUFS if mask_u8 else min(_STAGE_BUFS, 2)
    xT = nc.dram_tensor("xT", [D, N_LOC], F32, kind="ExternalInput").ap()
    mxT = nc.dram_tensor("mxT", [D, N_LOC], MDT, kind="ExternalInput").ap()
    yT = nc.dram_tensor("yT", [D, M_LOC], F32, kind="ExternalInput").ap()
    myT = nc.dram_tensor("myT", [D, M_LOC], MDT, kind="ExternalInput").ap()
    wT = nc.dram_tensor("wT", [D, D], F32, kind="ExternalInput").ap()
    sc = nc.dram_tensor("sc", [P, 1], F32, kind="ExternalInput").ap()
    odt = {
        "bfloat16": mybir.dt.bfloat16,
        "float16": mybir.dt.float16,
    }.get(OUT_DT, F32)
    out = nc.dram_tensor("out", [N_LOC, M_LOC], odt, kind="ExternalOutput").ap()

    sd1 = _store_dt(MM1_DT)
    sd2 = _store_dt(MM2_DT)

    with tile.TileContext(nc) as tc:
        with (
            tc.tile_pool(name="const", bufs=1) as const_pool,
            tc.tile_pool(name="persist", bufs=1) as persist,
            tc.tile_pool(name="stage", bufs=stage_bufs) as stage,
            tc.tile_pool(name="outp", bufs=outp_bufs) as outp,
        ):
            # sigmoid scale (dropout (1/keep)^2 folded out of the u8 masks)
            sct = const_pool.tile([P, 1], F32, name="sct")
            nc.sync.dma_start(out=sct[:], in_=sc[:])
            # W^T resident: [din-part, dc, kout]; DMA to a staging tile, then
            # a DVE copy that rounds to the matmul dtype (fp32r needs
            # explicit rounding; plain copy otherwise).
            wt = const_pool.tile([P, DC, D], sd1, name="wt")
            for dc in range(DC):
                wt_raw = stage.tile([P, D], F32, name="wt_raw", tag="sx")
                nc.sync.dma_start(
                    out=wt_raw[:], in_=wT[dc * P : (dc + 1) * P, :]
                )
                nc.vector.tensor_copy(
                    out=_round_out(wt[:, dc, :], MM1_DT), in_=wt_raw[:]
                )

            # Persistent operand tiles. Masks arrive as uint8 {0,1}; the
            # dropped 1/keep scaling (1.25 per side) is folded into the final
            # sigmoid's scale argument.
            xdt = persist.tile([P, DC, N_LOC], sd1, name="xdt")
            ydt = persist.tile([P, DC, M_LOC], sd2, name="ydt")
            xtt = persist.tile([P, DC, N_LOC], sd2, name="xtt")

            # Both PSUM pools open concurrently (2 + 3*2 = 8 banks) so
            # matmul2's first tile doesn't wait on matmul1's pool release.
            ctx_psum1 = tc.tile_pool(
                name="psum1",
                bufs=int(os.environ.get("PSUM1_BUFS", "2")),
                space="PSUM",
            )
            psum1 = ctx_psum1.__enter__()
            psum2_ctx = tc.tile_pool(name="psum2", bufs=_PSUM2_BUFS, space="PSUM")
            psum2 = psum2_ctx.__enter__()

            def load_x_nt(nt):
                # column chunk nt of xdT across all 4 d-chunks; x data lands
                # directly in the persistent tile, mask-multiply is in place.
                sl = slice(nt * 512, (nt + 1) * 512)
                for dc in range(DC):
                    sm = stage.tile([P, 512], MDT, name="sm", tag="sm")
                    nc.sync.dma_start(
                        out=_round_out(xdt[:, dc, sl], MM1_DT),
                        in_=_round_out(xT[dc * P : (dc + 1) * P, sl], MM1_DT),
                    )
                    nc.sync.dma_start(out=sm[:], in_=mxT[dc * P : (dc + 1) * P, sl])
                    nc.vector.tensor_mul(
                        out=_round_out(xdt[:, dc, sl], MM1_DT),
                        in0=xdt[:, dc, sl],
                        in1=sm[:],
                    )

            def load_y_q(q, width=2048):
                sl = slice(q * width, (q + 1) * width)
                for dc in range(DC):
                    smy = stage.tile([P, width], MDT, name="smy", tag="smy")
                    nc.sync.dma_start(
                        out=_round_out(ydt[:, dc, sl], MM2_DT),
                        in_=_round_out(yT[dc * P : (dc + 1) * P, sl], MM2_DT),
                    )
                    nc.sync.dma_start(out=smy[:], in_=myT[dc * P : (dc + 1) * P, sl])
                    nc.vector.tensor_mul(
                        out=_round_out(ydt[:, dc, sl], MM2_DT),
                        in0=ydt[:, dc, sl],
                        in1=smy[:],
                    )

            def mm1_nt(nt):
                # xtT[k, nt-cols] += WT[d,k].T @ xdT[d, nt-cols]
                for kc in range(DC):
                    ps = psum1.tile([P, 512], F32, name="ps1")
                    for dc in range(DC):
                        nc.tensor.matmul(
                            ps[:],
                            lhsT=_mm_view(wt[:, dc, kc * P : (kc + 1) * P], MM1_DT),
                            rhs=_mm_view(
                                xdt[:, dc, nt * 512 : (nt + 1) * 512], MM1_DT
                            ),
                            start=(dc == 0),
                            stop=(dc == DC - 1),
                        )
                    nc.vector.tensor_copy(
                        out=_round_out(
                            xtt[:, kc, nt * 512 : (nt + 1) * 512], MM2_DT
                        ),
                        in_=ps[:],
                    )

            # matmul2 + sigmoid + store for one [128n x MB] PSUM block
            MB = _MB

            def mm2_tile(mb, nchunk):
                ps = psum2.tile([P, MB], F32, name="ps2")
                for kc in range(DC):
                    for mt in range(MB // 512):
                        nc.tensor.matmul(
                            ps[:, mt * 512 : (mt + 1) * 512],
                            lhsT=_mm_view(
                                xtt[:, kc, nchunk * P : (nchunk + 1) * P], MM2_DT
                            ),
                            rhs=_mm_view(
                                ydt[
                                    :,
                                    kc,
                                    mb * MB + mt * 512 : mb * MB + (mt + 1) * 512,
                                ],
                                MM2_DT,
                            ),
                            start=(kc == 0),
                            stop=(kc == DC - 1),
                        )
                sig = outp.tile([P, MB], odt, name="sig")
                nc.scalar.activation(
                    sig[:], ps[:], mybir.ActivationFunctionType.Sigmoid,
                    scale=sct[:],
                )
                if not _ABL_NO_OUT:
                    nc.sync.dma_start(
                        out=out[
                            nchunk * P : (nchunk + 1) * P, mb * MB : (mb + 1) * MB
                        ],
                        in_=sig[:],
                    )

            # Emission order sets scheduler priority: x-nt0 first (unblocks
            # matmul1 -> xtt), y-h0 next (unblocks matmul2's stream), then
            # the remaining loads pipelined with matmul1 column-chunks;
            # matmul2 last.
            # With bf16 output the DMA device has slack, so matmul2 starts as
            # early as its data allows: a quarter-width first y chunk gates
            # mm2-mb0 at ~14us, and mm2 nchunk-groups interleave with the
            # remaining loads / matmul1 column-chunks to keep the PE fed.
            if _ABL_NO_MM2:
                load_x_nt(0)
                load_y_q(0, 1024)
                load_y_q(1, 1024)
                mm1_nt(0)
                for nt in range(1, N_LOC // 512):
                    load_x_nt(nt)
                    mm1_nt(nt)
                load_y_q(1, 2048)
            else:
                load_x_nt(0)
                load_y_q(0, 1024)
                mm1_nt(0)
                for g in range(1, 4):
                    load_x_nt(g)
                    if g == 1:
                        load_y_q(1, 1024)
                    for nchunk in range((g - 1) * 4, g * 4):
                        mm2_tile(0, nchunk)
                    mm1_nt(g)
                load_y_q(1, 2048)
                for nchunk in range(12, 16):
                    mm2_tile(0, nchunk)
                for mb in range(1, M_LOC // MB):
                    for nchunk in range(N_LOC // P):
                        mm2_tile(mb, nchunk)
            psum2_ctx.__exit__(None, None, None)
            ctx_psum1.__exit__(None, None, None)

    nc.compile()
    return nc


_NC = {}


def _get_nc(mask_u8: bool = True):
    if mask_u8 not in _NC:
        _NC[mask_u8] = _build(mask_u8)
    return _NC[mask_u8]


def _two_valued(mask):
    """(is_two_valued {0, c}, c) — True for inverted-dropout masks."""
    c = float(mask.max())
    ok = bool(np.all((mask == 0) | (mask == np.float32(c))))
    return ok, c


def kernel(x, y, mask_x, mask_y, W):
    x = np.asarray(x, dtype=np.float32)
    y = np.asarray(y, dtype=np.float32)
    mask_x = np.asarray(mask_x, dtype=np.float32)
    mask_y = np.asarray(mask_y, dtype=np.float32)
    W = np.asarray(W, dtype=np.float32)

    okx, cx = _two_valued(mask_x)
    oky, cy = _two_valued(mask_y)
    mask_u8 = okx and oky
    if mask_u8:
        mxT = np.ascontiguousarray((mask_x.T != 0).astype(np.uint8))
        myT = np.ascontiguousarray((mask_y.T != 0).astype(np.uint8))
        scale = np.float32(cx) * np.float32(cy)
    else:
        mxT = np.ascontiguousarray(mask_x.T)
        myT = np.ascontiguousarray(mask_y.T)
        scale = np.float32(1.0)
    sc = np.full((P, 1), scale, dtype=np.float32)

    xT = np.ascontiguousarray(x.T)
    yT = np.ascontiguousarray(y.T)
    wT = np.ascontiguousarray(W.T)

    in_maps = []
    for c in range(8):
        i, j = c // GRID_M, c % GRID_M
        in_maps.append(
            {
                "xT": np.ascontiguousarray(xT[:, i * N_LOC : (i + 1) * N_LOC]),
                "mxT": np.ascontiguousarray(mxT[:, i * N_LOC : (i + 1) * N_LOC]),
                "yT": np.ascontiguousarray(yT[:, j * M_LOC : (j + 1) * M_LOC]),
                "myT": np.ascontiguousarray(myT[:, j * M_LOC : (j + 1) * M_LOC]),
                "wT": wT,
                "sc": sc,
            }
        )

    res = run_bass_kernel_spmd(_get_nc(mask_u8), in_maps, list(range(8)))

    out = np.empty((N, M), dtype=np.float32)
    for c in range(8):
        i, j = c // GRID_M, c % GRID_M
        out[i * N_LOC : (i + 1) * N_LOC, j * M_LOC : (j + 1) * M_LOC] = (
            res.results[c]["out"].astype(np.float32)
        )
    return out



# revision 2
# speedup vs baseline: 1.1395x; 1.1395x over previous
"""Trainium2 Bass kernel for BilinearDiscriminator.

Computes sigmoid((x*mask_x) @ W.T @ (y*mask_y).T) for x,y [8192,512],
W [512,512] -> out [8192,8192] fp32, SPMD across 8 NeuronCores.

Sharding: 8x1 row-parallel (the 4x2 grid duplicated mm1 on every
m-column; 8x1 halves mm1's PE work). Core c handles rows
[c*1024, (c+1)*1024) of x; W and y are replicated. Host pre-applies the
dropout masks and casts everything to fp16 (same 1 cyc/row PE speed as
fp32r but half the DMA bytes and no on-chip mask multiplies; measured
rel err of the full-fp16 chain is ~8e-4 vs the 2e-2 gate).

Per-core schedule, built around keeping the PE continuously busy:
  - warm-up: a memset tile feeds a few dummy matmuls so the PE p-state
    ramps while the first input DMAs are in flight.
  - mm1 (xt^T = W^T-chunks @ xd^T) runs dc-outer for the first column
    block so compute starts as soon as the first (W, xd) d-chunks land.
  - mm2 streams [128n x 1024m] PSUM tiles (4 k-chunks x 2 bank-halves),
    ACT applies sigmoid PSUM->fp16 SBUF, DMA writes out. The last tile
    is split 2x512 to shorten the post-last-matmul tail.
"""

import os
import sys

sys.path.insert(0, "/opt/trn_rl_repo")

import numpy as np

import concourse.bass as bass
import concourse.mybir as mybir
import concourse.tile as tile
from concourse import bacc
from concourse.bass_utils import run_bass_kernel_spmd

P = 128
N, M, D = 8192, 8192, 512
GRID = 8
N_LOC = N // GRID  # 1024
DC = D // P  # 4 chunks of the contraction dims
MB = 1024  # mm2 column block (2 PSUM banks)

F16 = mybir.dt.float16
F32 = mybir.dt.float32

_N_WARM = int(os.environ.get("N_WARM", "6"))
_WARM_ROWS0 = int(os.environ.get("WARM_ROWS0", "128"))
_SIG_BUFS = int(os.environ.get("SIG_BUFS", "12"))
_PSUM1_BUFS = int(os.environ.get("PSUM1_BUFS", "4"))
_PSUM2_BUFS = int(os.environ.get("PSUM2_BUFS", "2"))
_TAIL_SPLIT = int(os.environ.get("TAIL_SPLIT", "512"))


def _build():
    nc = bacc.Bacc("TRN2", target_bir_lowering=False, debug=False)

    xdT = nc.dram_tensor("xdT", [D, N_LOC], F16, kind="ExternalInput").ap()
    wT = nc.dram_tensor("wT", [D, D], F16, kind="ExternalInput").ap()
    ydT = nc.dram_tensor("ydT", [D, M], F16, kind="ExternalInput").ap()
    out = nc.dram_tensor("out", [N_LOC, M], F16, kind="ExternalOutput").ap()

    # [ (dc p) cols ] -> [ p dc cols ] so one DMA can fill a multi-d-chunk
    # SBUF tile slice in partition-major order.
    wT_r = wT.rearrange("(dc p) k -> p dc k", p=P)
    xdT_r = xdT.rearrange("(dc p) n -> p dc n", p=P)
    ydT_r = ydT.rearrange("(dc p) m -> p dc m", p=P)

    with tile.TileContext(nc) as tc:
        with (
            tc.tile_pool(name="persist", bufs=1) as persist,
            tc.tile_pool(name="sig", bufs=_SIG_BUFS) as sigp,
        ):
            warm = persist.tile([P, 512], F16, name="warm")
            nc.vector.memset(warm[:], 0.0)

            wt = persist.tile([P, DC, D], F16, name="wt")
            xdt = persist.tile([P, DC, N_LOC], F16, name="xdt")
            ydt = persist.tile([P, DC, M], F16, name="ydt")
            xtt = persist.tile([P, DC, N_LOC], F16, name="xtt")

            ctx_psum1 = tc.tile_pool(name="psum1", bufs=_PSUM1_BUFS, space="PSUM")
            psum1 = ctx_psum1.__enter__()
            ctx_psum2 = tc.tile_pool(name="psum2", bufs=_PSUM2_BUFS, space="PSUM")
            psum2 = ctx_psum2.__enter__()

            # PE p-state warm-up: dependency-free dummy matmuls fill the DMA
            # lead-in so real matmuls start at full clock. They rotate through
            # the psum2 pool (idle until mm2) to avoid stealing psum1 bufs.
            for i in range(_N_WARM):
                pw = psum2.tile([P, MB], F32, name="ps2")
                rows = _WARM_ROWS0 if i == 0 else 512
                nc.tensor.matmul(
                    pw[:, :rows],
                    lhsT=warm[:, :P],
                    rhs=warm[:, :rows],
                    start=True,
                    stop=True,
                )

            # Input DMAs, in priority order. First (W, xd) d-chunks are split
            # out so mm1 can start ~1us earlier; y streams in mb-order.
            nc.sync.dma_start(out=wt[:, 0:2, :], in_=wT_r[:, 0:2, :])
            nc.sync.dma_start(out=xdt[:, 0:2, 0:512], in_=xdT_r[:, 0:2, 0:512])
            nc.sync.dma_start(out=wt[:, 2:4, :], in_=wT_r[:, 2:4, :])
            nc.sync.dma_start(out=xdt[:, 2:4, 0:512], in_=xdT_r[:, 2:4, 0:512])
            nc.sync.dma_start(out=xdt[:, :, 512:1024], in_=xdT_r[:, :, 512:1024])
            for mb in range(M // MB):
                nc.sync.dma_start(
                    out=ydt[:, :, mb * MB : (mb + 1) * MB],
                    in_=ydT_r[:, :, mb * MB : (mb + 1) * MB],
                )

            # mm1, nt0 (cols 0:512), dc-outer so the first 8 matmuls need only
            # d-chunks 0-1; all 4 kc accumulators live in psum1 at once.
            ps1 = [psum1.tile([P, 512], F32, name="ps1") for _ in range(DC)]
            for dc in range(DC):
                for kc in range(DC):
                    nc.tensor.matmul(
                        ps1[kc][:],
                        lhsT=wt[:, dc, kc * P : (kc + 1) * P],
                        rhs=xdt[:, dc, 0:512],
                        start=(dc == 0),
                        stop=(dc == DC - 1),
                    )
            for kc in range(DC):
                nc.vector.tensor_copy(out=xtt[:, kc, 0:512], in_=ps1[kc][:])

            # mm1, nt1 (cols 512:1024), kc-outer (all data resident by now).
            for kc in range(DC):
                ps = psum1.tile([P, 512], F32, name="ps1")
                for dc in range(DC):
                    nc.tensor.matmul(
                        ps[:],
                        lhsT=wt[:, dc, kc * P : (kc + 1) * P],
                        rhs=xdt[:, dc, 512:1024],
                        start=(dc == 0),
                        stop=(dc == DC - 1),
                    )
                nc.vector.tensor_copy(out=xtt[:, kc, 512:1024], in_=ps[:])

            # mm2 + sigmoid + store, streaming mb-major.
            n_mb = M // MB
            n_nc = N_LOC // P

            def mm2_tile(mb, nchunk, split=None):
                ps = psum2.tile([P, MB], F32, name="ps2")
                for kc in range(DC):
                    for mt in range(MB // 512):
                        nc.tensor.matmul(
                            ps[:, mt * 512 : (mt + 1) * 512],
                            lhsT=xtt[:, kc, nchunk * P : (nchunk + 1) * P],
                            rhs=ydt[
                                :, kc, mb * MB + mt * 512 : mb * MB + (mt + 1) * 512
                            ],
                            start=(kc == 0),
                            stop=(kc == DC - 1),
                        )
                widths = [MB] if not split else [split] * (MB // split)
                off = 0
                for w in widths:
                    sig = sigp.tile([P, MB], F16, name="sig")
                    nc.scalar.activation(
                        sig[:, :w],
                        ps[:, off : off + w],
                        mybir.ActivationFunctionType.Sigmoid,
                    )
                    nc.sync.dma_start(
                        out=out[
                            nchunk * P : (nchunk + 1) * P,
                            mb * MB + off : mb * MB + off + w,
                        ],
                        in_=sig[:, :w],
                    )
                    off += w

            for mb in range(n_mb):
                for nchunk in range(n_nc):
                    last = mb == n_mb - 1 and nchunk == n_nc - 1
                    mm2_tile(mb, nchunk, split=_TAIL_SPLIT if last else None)

            ctx_psum2.__exit__(None, None, None)
            ctx_psum1.__exit__(None, None, None)

    nc.compile()
    return nc


_NC = {}


def _get_nc():
    if "nc" not in _NC:
        _NC["nc"] = _build()
    return _NC["nc"]


def kernel(x, y, mask_x, mask_y, W):
    x = np.asarray(x, dtype=np.float32)
    y = np.asarray(y, dtype=np.float32)
    mask_x = np.asarray(mask_x, dtype=np.float32)
    mask_y = np.asarray(mask_y, dtype=np.float32)
    W = np.asarray(W, dtype=np.float32)

    xdT = np.ascontiguousarray((x * mask_x).T.astype(np.float16))
    ydT = np.ascontiguousarray((y * mask_y).T.astype(np.float16))
    wT = np.ascontiguousarray(W.T.astype(np.float16))

    in_maps = []
    for c in range(GRID):
        in_maps.append(
            {
                "xdT": np.ascontiguousarray(xdT[:, c * N_LOC : (c + 1) * N_LOC]),
                "wT": wT,
                "ydT": ydT,
            }
        )

    res = run_bass_kernel_spmd(_get_nc(), in_maps, list(range(8)))

    out = np.empty((N, M), dtype=np.float32)
    for c in range(GRID):
        out[c * N_LOC : (c + 1) * N_LOC, :] = res.results[c]["out"].astype(
            np.float32
        )
    return out
